# revision 32
# baseline (speedup 1.0000x reference)
"""Distributed Trainium2 Bass kernel for multi-head causal cross-attention.

Reference computation (B=2, T=2048, E=1024, H=16, d=64):
    q = x @ Wq + bq ; k = y @ Wk + bk ; v = y @ Wv + bv      (per-head reshape)
    att = softmax(q k^T / sqrt(d) + causal_mask)
    out = (att v) @ Wo + bo

Sharding over 8 NeuronCores: data-parallel on batch (2 groups of 4 cores),
tensor-parallel on heads (4 heads = 256 channels per core).  Each core
computes a partial output projection; the host sums the 4 partials per batch
(the unshard step for tensor-parallel partial sums) and adds the output bias.
No on-device collectives are needed.

Per-core dataflow (bf16 operands, fp32 PSUM accumulation; measured
rel err ~5.4e-3 vs the fp32 reference):
  - host passes x^T / y^T (bf16) pre-blocked by 512-col t-blocks so every
    input DMA reads 8KB/partition contiguous runs; no on-chip transposes
  - Q^T,K^T = W^T x^T (W stationary), evicted bf16 with fused bias add
  - V in an augmented layout [tk, 4*65]: per head 64 value columns plus a
    ones column, so the PV matmul (M=65) also emits the softmax denominator
    as PSUM row 64
  - scores computed transposed (S^T: tk on partitions, tq free) into a
    2-bank PSUM tile holding both heads of a pair (the two 64-row score
    matmuls run CONCURRENTLY via PE row-group tiling); causal blocks
    skipped; one exp (scale=1/8 fused, no max-subtraction) covers both
    heads via a segmented AP; diagonal 128-blocks masked with one
    segmented 0/1 triangular multiply on the DVE
  - normalization: fast approximate reciprocal of the sums row + gpsimd
    partition-broadcast + fused multiply while evicting A^T
  - one global chunk stream over (J, pair, tk-chunk) with a one-chunk
    score LOOKAHEAD: chunk c+1's score matmuls are queued before exp(c) so
    the scalar engine (the pacer, ~93us of exp) never waits; deferred
    projection work (K/Q/V for J+1, outproj) fills the tensor engine
    between chunks, rebalanced toward the late exp-paced streams
  - PSUM: 2 x 2-bank score slots + 1-bank deferred-work slot x2 + 2
    PV-accumulator banks = 8

Hardware notes baked in (learned from profiling):
  - bf16 moving operands stream 1 col/cycle (2.0-2.4 GHz, power-state
    dependent); all matmul operands bf16, fp32 PSUM accumulation
  - ACT: ~(N+352)/1.2 ns per activation; the one-time exp table load
    (~2.7us) is pulled off the critical path by an early dummy exp
  - HWDGE DMA trigger instructions BLOCK the issuing engine for the whole
    transfer: scalar queue only carries pre-exp loads; sync carries the
    rest; SWDGE (~38 GB/s) only wo
  - gpsimd cannot read PSUM; reciprocal_approx_fast needs an SBUF source
  - HAM clock gate: dense MM stream from first MM on keeps the PE at
    2.4 GHz; any >=3.4us tensor idle window halves the clock
"""

import sys

if "/opt/trn_rl_repo" not in sys.path:
    sys.path.insert(0, "/opt/trn_rl_repo")

import numpy as np
import ml_dtypes

import concourse.bacc as bacc
import concourse.mybir as mybir
import concourse.tile as tile
from concourse.bass_utils import run_bass_kernel_spmd

BF16 = mybir.dt.bfloat16
F32 = mybir.dt.float32
AF = mybir.ActivationFunctionType

B, T, E, H = 2, 2048, 1024, 16
D = E // H                  # 64 head dim
N_CORES = 8
CPC = E // 4                # 256 channels per core (4 heads)
NEG = -1.0e10

_CACHE = {}
LAST_RESULT = None


def _build():
    nc = bacc.Bacc("TRN2", target_bir_lowering=False, debug=False, num_devices=N_CORES)

    # xt/yt are host-pre-blocked: row (J*128 + p), col (j*512 + f) holds
    # x^T[j*128 + p, 512*J + f] -- each 512-col t-block is a contiguous
    # 8 KB/partition DRAM read (1 KB runs measured only ~125 GB/s)
    xt = nc.dram_tensor("xt", [512, 4096], BF16, kind="ExternalInput").ap()
    yt = nc.dram_tensor("yt", [512, 4096], BF16, kind="ExternalInput").ap()
    wq = nc.dram_tensor("wq", [128, 8 * CPC], BF16, kind="ExternalInput").ap()
    wk = nc.dram_tensor("wk", [128, 8 * CPC], BF16, kind="ExternalInput").ap()
    wvaug = nc.dram_tensor("wvaug", [128, 8 * 260], BF16, kind="ExternalInput").ap()
    wo = nc.dram_tensor("wo", [128, 2 * E], BF16, kind="ExternalInput").ap()
    # packed consts: cbf = [btri2 (256) | row0: bvaug (260)], cf32 = [bq0,
    # bq1, bk0, bk1] columns -- one DMA trigger each (HWDGE triggers cost
    # ~700ns of engine time apiece)
    cbf = nc.dram_tensor("cbf", [128, 516], BF16, kind="ExternalInput").ap()
    cf32 = nc.dram_tensor("cf32", [128, 4], F32, kind="ExternalInput").ap()
    out = nc.dram_tensor("out", [T, E], BF16, kind="ExternalOutput").ap()

    with tile.TileContext(nc) as tc:
        with (
            nc.allow_low_precision(reason="f32r intermediates; verified <2e-2 end-to-end"),
            tc.tile_pool(name="big", bufs=1) as big,
            tc.tile_pool(name="pt", bufs=8) as ptp,
            tc.tile_pool(name="small", bufs=3) as sm,
            tc.tile_pool(name="zout", bufs=4) as zp,
        ):
            # ---- inputs stream t-block-first on the two HWDGE queues
            # (sync: yT + wv; scalar: wk/wq + xT + wo) so the first K matmul
            # can start as soon as wk + yT block 0 land (~14us); weights ride
            # HWDGE too -- SWDGE measured only ~38 GB/s.  SWDGE keeps just
            # the tiny consts. ----
            wk_b = big.tile([128, 8 * CPC], BF16, tag="wk_b", name="wk_b")
            wv_b = big.tile([128, 8 * 260], BF16, tag="wv_b", name="wv_b")
            wq_b = big.tile([128, 8 * CPC], BF16, tag="wq_b", name="wq_b")
            wo_b = big.tile([128, 2 * E], BF16, tag="wo_b", name="wo_b")
            yTb = big.tile([128, 8 * T], BF16, tag="yTb", name="yTb")
            xTb = big.tile([128, 8 * T], BF16, tag="xTb", name="xTb")

            def in_blk(dst, src, J):
                return (
                    dst[:].rearrange("p (j f) -> p j f", j=8)[
                        :, :, 512 * J : 512 * J + 512
                    ],
                    src[128 * J : 128 * J + 128, :].rearrange(
                        "p (j f) -> p j f", j=8
                    ),
                )

            # HWDGE DMA triggers BLOCK the issuing engine for the whole
            # transfer (measured), so the scalar engine only carries loads
            # that finish before the first exp (~20us); everything later
            # rides sync (idle) or SWDGE (wo: slow queue but time-rich).
            # Tiny consts go FIRST on scalar -- the V-bias broadcast gated
            # the whole V pipeline when they sat behind SWDGE.
            btri2_t = big.tile([128, 256], BF16, tag="btri2", name="btri2")
            bvaug_t = big.tile([1, 260], BF16, tag="bvaug", name="bvaug")
            cf32_t = big.tile([128, 4], F32, tag="cf32", name="cf32")
            bq_t = [cf32_t[:, p : p + 1] for p in range(2)]
            bk_t = [cf32_t[:, 2 + p : 3 + p] for p in range(2)]
            # scalar queue: wk + wq (+2 tiny consts) only -- everything is
            # done by ~17us, before the first exp.  sync: yT0, xT0 (Q before
            # V: the chunk stream starts right after Q p0), then wv and the
            # remaining blocks.  SWDGE: btri2 (needed ~21us) + wo (~95us).
            xb0 = in_blk(xTb, xt, 0)
            yb0 = in_blk(yTb, yt, 0)
            nc.scalar.dma_start(wk_b[:, 0:1024], wk[:, 0:1024])
            nc.sync.dma_start(yb0[0][:, 0:4], yb0[1][:, 0:4])
            nc.scalar.dma_start(bvaug_t[:], cbf[0:1, 256:516])
            nc.scalar.dma_start(cf32_t[:], cf32[:, :])
            nc.gpsimd.dma_start(btri2_t[:], cbf[:, 0:256])
            nc.scalar.dma_start(wk_b[:, 1024:2048], wk[:, 1024:2048])
            nc.sync.dma_start(yb0[0][:, 4:8], yb0[1][:, 4:8])
            nc.scalar.dma_start(wq_b[:, 0:1024], wq[:, 0:1024])
            nc.sync.dma_start(xb0[0][:, 0:4], xb0[1][:, 0:4])
            nc.scalar.dma_start(wq_b[:, 1024:2048], wq[:, 1024:2048])
            nc.sync.dma_start(xb0[0][:, 4:8], xb0[1][:, 4:8])
            # dummy activation on a scratch tile: pulls the one-time
            # ACT_TABLE_LOAD (~2.7us) off the first real exp; last thing the
            # scalar engine does before the exp stream
            dum_t = sm.tile([1, 16], F32, tag="dum", name="dum")
            nc.scalar.activation(dum_t[:], dum_t[:], AF.Exp, scale=0.125)

            # PE pre-warm: ~20 dep-free matmuls on a memset scratch tile fill
            # the otherwise-idle DMA window (~3.5-12.8us) with PE activity so
            # the HAM clock gate flips to 2.4 GHz BEFORE the real matmuls
            # start (saves ~4us of half-clock execution)
            dum2 = big.tile([128, 512], BF16, tag="dum2", name="dum2")
            nc.vector.memset(dum2[:], 0.0)
            nc.sync.dma_start(wv_b[:, 0:1040], wvaug[:, 0:1040])
            nc.sync.dma_start(wv_b[:, 1040:2080], wvaug[:, 1040:2080])
            for J in range(1, 4):
                nc.sync.dma_start(*in_blk(yTb, yt, J))
                nc.sync.dma_start(*in_blk(xTb, xt, J))
            nc.gpsimd.dma_start(wo_b[:], wo[:, :])

            wk_t = [wk_b[:, CPC * e : CPC * e + CPC] for e in range(8)]
            wv_t = [wv_b[:, 260 * e : 260 * e + 260] for e in range(8)]
            wq_t = [wq_b[:, CPC * e : CPC * e + CPC] for e in range(8)]
            wo_t = [wo_b[:, E * p : E * p + E] for p in range(2)]
            yT = [yTb[:, T * e : T * e + T] for e in range(8)]
            xT = [xTb[:, T * e : T * e + T] for e in range(8)]

            # V-bias broadcast (consts loaded early on the scalar queue)
            vb_bc = big.tile([128, 260], BF16, tag="vb_bc", name="vb_bc")
            nc.gpsimd.partition_broadcast(vb_bc[:], bvaug_t[0:1, :])

            KT = [big.tile([128, T], BF16, tag=f"KT{p}", name=f"KT{p}") for p in range(2)]
            QT = [big.tile([128, T], BF16, tag=f"QT{p}", name=f"QT{p}") for p in range(2)]
            AT = [big.tile([128, T], BF16, tag=f"AT{p}", name=f"AT{p}") for p in range(2)]
            V = [big.tile([128, 260], BF16, tag=f"V{c}", name=f"V{c}") for c in range(16)]

            # ---- fused pipeline over tq-blocks J ----
            # Per J: attention chunk loop for both pairs, with the non-exp PE
            # work (K^T/Q^T/V production for J+1, output projection for J-1)
            # interleaved between chunks so the scalar engine (exp) never
            # starves.  J=0's own QKV is a prelude; J=3's outproj is a tail.
            with tc.tile_pool(name="psa", bufs=2, space="PSUM") as psa:
                dups = psa.tile([65, 512], F32, tag="a0", bufs=1, name="dups")
                for _ in range(20):
                    nc.tensor.matmul(
                        dups[:],
                        dum2[:, 0:65],
                        dum2[:, 0:512],
                        start=True,
                        stop=True,
                    )

                # deferred work uses its own 1-bank PSUM tag "w0" so it never
                # rotates through the score slots (sharing collapsed the
                # score lookahead and bubbled the exp stream); emit_kq is
                # split into two 4-matmul halves so work quanta stay under
                # ~1us between score/PV emissions
                def emit_kq_half(J, p, which, half, st):
                    w_t, dst, bias, src_t = (
                        (wk_t, KT, bk_t, yT) if which == "k" else (wq_t, QT, bq_t, xT)
                    )
                    if half == 0:
                        st["ps"] = psa.tile(
                            [128, 512], F32, tag="w0", bufs=2, name="qk"
                        )
                    ps = st["ps"]
                    for e in range(4 * half, 4 * half + 4):
                        nc.tensor.matmul(
                            ps[:],
                            w_t[e][:, 128 * p : 128 * p + 128],
                            src_t[e][:, 512 * J : 512 * J + 512],
                            start=(e == 0),
                            stop=(e == 7),
                        )
                    if half == 1:
                        nc.vector.tensor_scalar_add(
                            dst[p][:, 512 * J : 512 * J + 512], ps[:], bias[p][:, 0:1]
                        )

                def emit_kq(J, p, which, tag="w0", bufs=2):
                    w_t, dst, bias, src_t = (
                        (wk_t, KT, bk_t, yT) if which == "k" else (wq_t, QT, bq_t, xT)
                    )
                    ps = psa.tile([128, 512], F32, tag=tag, bufs=bufs, name="qk")
                    for e in range(8):
                        nc.tensor.matmul(
                            ps[:],
                            w_t[e][:, 128 * p : 128 * p + 128],
                            src_t[e][:, 512 * J : 512 * J + 512],
                            start=(e == 0),
                            stop=(e == 7),
                        )
                    nc.vector.tensor_scalar_add(
                        dst[p][:, 512 * J : 512 * J + 512], ps[:], bias[p][:, 0:1]
                    )

                def emit_v(c):
                    psv = psa.tile([128, 260], F32, tag="w0", bufs=2, name="psv")
                    for e in range(8):
                        nc.tensor.matmul(
                            psv[:],
                            yT[e][:, 128 * c : 128 * c + 128],
                            wv_t[e][:],
                            start=(e == 0),
                            stop=(e == 7),
                        )
                    # bias + per-head ones columns fused into the eviction
                    nc.vector.tensor_add(V[c][:], psv[:], vb_bc[:])

                def emit_outproj(t, tail=False):
                    z = zp.tile([128, E], BF16, tag="z", name="z")
                    for eo in range(2):
                        pz = psa.tile([128, 512], F32, tag="w0", bufs=2, name="pz")
                        nc.tensor.matmul(
                            pz[:],
                            AT[0][:, 128 * t : 128 * t + 128],
                            wo_t[0][:, 512 * eo : 512 * eo + 512],
                            start=True,
                            stop=False,
                        )
                        nc.tensor.matmul(
                            pz[:],
                            AT[1][:, 128 * t : 128 * t + 128],
                            wo_t[1][:, 512 * eo : 512 * eo + 512],
                            start=False,
                            stop=True,
                        )
                        # mid-kernel: both halves on DVE (scalar paces the
                        # exp stream); tail: exp is done, so the idle scalar
                        # engine takes one half
                        if eo == 0:
                            nc.vector.tensor_copy(z[:, 0:512], pz[:])
                        elif tail:
                            nc.scalar.copy(z[:, 512:1024], pz[:])
                        else:
                            nc.vector.tensor_copy(z[:, 512:1024], pz[:])
                    # sync HWDGE: idle after the input load, and ~7x faster
                    # than SWDGE (the last block's eviction is the kernel
                    # tail)
                    nc.sync.dma_start(out[128 * t : 128 * t + 128, :], z[:])

                # prelude: QKV for J=0 -- K^T groups borrow the (still free)
                # PV-accumulator banks so more partial contractions can stay
                # open while the input DMAs drain; V (yT-dependent) before Q
                # so the xT block-0 DMA has time to land
                emit_kq(0, 0, "k", tag="a0", bufs=1)
                emit_kq(0, 1, "k", tag="a1", bufs=1)
                emit_kq(0, 0, "q")

                # ---- global chunk stream with one-chunk score lookahead:
                # chunk c+1's score matmuls are queued on the tensor engine
                # BEFORE exp(c), so the scalar engine (the pacer) never
                # stalls at pair/J boundaries; deferred projection work is
                # queued between them to fill the tensor engine during exp.
                acc = {}

                def emit_scores(J, p, i):
                    r = i - 4 * J
                    full = r < 0
                    lo = 0 if full else 128 * r
                    tqs = slice(512 * J + lo, 512 * J + 512)
                    s0 = psa.tile([128, 1024], F32, tag="s0", bufs=2, name="s0")
                    nc.tensor.matmul(
                        s0[:, lo:512],
                        KT[p][0:64, 128 * i : 128 * i + 128],
                        QT[p][0:64, tqs],
                        start=True,
                        stop=True,
                    )
                    nc.tensor.matmul(
                        s0[:, 512 + lo : 1024],
                        KT[p][64:128, 128 * i : 128 * i + 128],
                        QT[p][64:128, tqs],
                        start=True,
                        stop=True,
                    )
                    return (J, p, i, s0, lo, full)

                def normalize(J, p, o0, o1, c0=0, c1=512, tail=False):
                    # gpsimd cannot read PSUM, so the denominator rows stage
                    # through SBUF; approx recip also needs an SBUF source.
                    # On the very last pair the exp stream is over, so the
                    # idle scalar engine does the staging copies instead of
                    # the (tail-critical) DVE.
                    w = c1 - c0
                    ro0 = sm.tile([1, 512], F32, tag="ro0", name="ro0")
                    ro1 = sm.tile([1, 512], F32, tag="ro1", name="ro1")
                    if tail:
                        nc.scalar.copy(ro0[0:1, 0:w], o0[64:65, c0:c1])
                        nc.scalar.copy(ro1[0:1, 0:w], o1[64:65, c0:c1])
                    else:
                        nc.vector.tensor_copy(ro0[0:1, 0:w], o0[64:65, c0:c1])
                        nc.vector.tensor_copy(ro1[0:1, 0:w], o1[64:65, c0:c1])
                    re0 = sm.tile([1, 512], F32, tag="re0", name="re0")
                    re1 = sm.tile([1, 512], F32, tag="re1", name="re1")
                    nc.vector.reciprocal_approx_fast(re0[0:1, 0:w], ro0[0:1, 0:w])
                    nc.vector.reciprocal_approx_fast(re1[0:1, 0:w], ro1[0:1, 0:w])
                    bs0 = sm.tile([64, 512], F32, tag="bs0", name="bs0")
                    bs1 = sm.tile([64, 512], F32, tag="bs1", name="bs1")
                    nc.gpsimd.partition_broadcast(bs0[0:64, 0:w], re0[0:1, 0:w])
                    nc.gpsimd.partition_broadcast(bs1[0:64, 0:w], re1[0:1, 0:w])
                    Js = slice(512 * J + c0, 512 * J + c1)
                    nc.vector.tensor_mul(
                        AT[p][0:64, Js], o0[0:64, c0:c1], bs0[0:64, 0:w]
                    )
                    nc.vector.tensor_mul(
                        AT[p][64:128, Js], o1[0:64, c0:c1], bs1[0:64, 0:w]
                    )

                def process(ch):
                    J, p, i, s0, lo, full = ch
                    nchunks = 4 * J + 4
                    pt0 = ptp.tile([128, 1024], BF16, tag="pt0", name="pt0")
                    if full:
                        nc.scalar.activation(pt0[:], s0[:], AF.Exp, scale=0.125)
                    else:
                        s3 = s0[:].rearrange("p (s f) -> p s f", s=2)[:, :, lo:512]
                        p3 = pt0[:].rearrange("p (s f) -> p s f", s=2)[:, :, lo:512]
                        nc.scalar.activation(p3, s3, AF.Exp, scale=0.125)
                        # causal band: zero weights where tk > tq -- both
                        # heads' diagonal blocks in ONE segmented multiply
                        pdg = pt0[:].rearrange("p (s f) -> p s f", s=2)[
                            :, :, lo : lo + 128
                        ]
                        nc.vector.tensor_mul(
                            pdg,
                            pdg,
                            btri2_t[:].rearrange("p (s f) -> p s f", s=2),
                        )
                    if i == 0:
                        acc[p] = (
                            psa.tile([65, 512], F32, tag="a0", bufs=1, name="a0"),
                            psa.tile([65, 512], F32, tag="a1", bufs=1, name="a1"),
                        )
                    o0, o1 = acc[p]
                    h0 = 65 * (2 * p)
                    h1 = 65 * (2 * p + 1)
                    nc.tensor.matmul(
                        o0[0:65, lo:512],
                        V[i][:, h0 : h0 + 65],
                        pt0[:, lo:512],
                        start=(i == 0),
                        stop=(i == nchunks - 1),
                    )
                    nc.tensor.matmul(
                        o1[0:65, lo:512],
                        V[i][:, h1 : h1 + 65],
                        pt0[:, 512 + lo : 1024],
                        start=(i == 0),
                        stop=(i == nchunks - 1),
                    )
                    if i == nchunks - 1:
                        if J == 3 and p == 1:
                            # tail: normalize in column halves so the final
                            # outproj blocks overlap the second half
                            normalize(J, p, o0, o1, 0, 256, tail=True)
                            # 10/11 have no new deps: their matmuls fill the
                            # tensor engine during the normalize chain
                            emit_outproj(10, tail=True)
                            emit_outproj(11, tail=True)
                            emit_outproj(12, tail=True)
                            emit_outproj(13, tail=True)
                            normalize(J, p, o0, o1, 256, 512, tail=True)
                            emit_outproj(14, tail=True)
                            emit_outproj(15, tail=True)
                        else:
                            normalize(J, p, o0, o1)

                pend = None
                for J in range(4):
                    # deferred work: QKV for J+1 when the data allows it;
                    # outproj rebalanced toward the exp-paced late streams
                    # (J=2: blocks 0-3, J=3: blocks 4-11) where the tensor
                    # engine otherwise starves
                    work = []
                    # J=0: its own V blocks and Q p1 lead the work list (PV
                    # and the pair-1 scores trail them by design)
                    if J == 0:
                        for c in range(0, 4):
                            work.append(lambda c=c: emit_v(c))
                        st0 = {}
                        for h in range(2):
                            work.append(
                                lambda h=h, st=st0: emit_kq_half(0, 1, "q", h, st)
                            )
                    # J=2 first produces its own diag-block V (consumed from
                    # chunk 8 onward; items 0-3 complete by ~chunk 5)
                    if J == 2:
                        for c in range(8, 12):
                            work.append(lambda c=c: emit_v(c))
                    if J < 3:
                        for p in range(2):
                            for w in ("k", "q"):
                                st = {}
                                for h in range(2):
                                    work.append(
                                        lambda p=p, J=J, w=w, h=h, st=st:
                                        emit_kq_half(J + 1, p, w, h, st)
                                    )
                    # v(4-7) stay in J0 (needed early in J1); v(8-11) moved
                    # into J2 above; v(12-15) stay in J2
                    if J == 0:
                        for c in range(4, 8):
                            work.append(lambda c=c: emit_v(c))
                    # v(12-15) are J3's diag blocks, consumed from chunk 12
                    # onward -- produced as J3's first work items (done by
                    # ~chunk 6); all movable outproj blocks also go to J3
                    if J == 3:
                        for c in range(12, 16):
                            work.append(lambda c=c: emit_v(c))
                        for t in range(0, 10):
                            work.append(lambda t=t: emit_outproj(t))

                    nchunks = 4 * J + 4
                    nw = len(work)
                    wi = 0
                    for p in range(2):
                        for i in range(nchunks):
                            ch = emit_scores(J, p, i)
                            # fill the tensor queue before the (dependent)
                            # PV of the previous chunk blocks it
                            hi_w = nw * (p * nchunks + i + 1) // (2 * nchunks)
                            while wi < hi_w:
                                work[wi]()
                                wi += 1
                            if pend is not None:
                                process(pend)
                            pend = ch
                process(pend)

    nc.compile()
    return nc


def _get_nc():
    if "nc" not in _CACHE:
        _CACHE["nc"] = _build()
    return _CACHE["nc"]


def _consts():
    if "consts" not in _CACHE:
        bf = ml_dtypes.bfloat16
        btri1 = (
            np.arange(128)[None, :] >= np.arange(128)[:, None]
        ).astype(np.float32).astype(bf)
        btri = np.ascontiguousarray(np.concatenate([btri1, btri1], axis=1))
        _CACHE["consts"] = (btri,)
    return _CACHE["consts"]


def kernel(
    x, y, mask, Wq, bq, Wk, bk, Wv, bv, Wo, bo, num_heads, trace=False
):
    global LAST_RESULT
    assert int(num_heads) == H
    x = np.asarray(x, dtype=np.float32)
    y = np.asarray(y, dtype=np.float32)
    Wq = np.asarray(Wq, dtype=np.float32)
    Wk = np.asarray(Wk, dtype=np.float32)
    Wv = np.asarray(Wv, dtype=np.float32)
    Wo = np.asarray(Wo, dtype=np.float32)
    bq = np.asarray(bq, dtype=np.float32)
    bk = np.asarray(bk, dtype=np.float32)
    bv = np.asarray(bv, dtype=np.float32)
    bo = np.asarray(bo, dtype=np.float32)

    bf = ml_dtypes.bfloat16
    (btri,) = _consts()

    def blk(a):
        # [T, E] -> transposed+t-blocked [4*128, 8*512]: row (J*128+p),
        # col (j*512+f) = a.T[j*128+p, 512*J+f]
        at = a.T.reshape(8, 128, 4, 512)
        return np.ascontiguousarray(
            at.transpose(2, 1, 0, 3).reshape(512, 4096)
        ).astype(bf)

    xtb = [blk(x[b]) for b in range(B)]
    ytb = [blk(y[b]) for b in range(B)]

    in_maps = []
    for c in range(N_CORES):
        b = c // 4
        g = c % 4
        cols = slice(CPC * g, CPC * g + CPC)
        wv_s = Wv[:, cols]
        bv_s = bv[cols]
        wvaug = np.zeros((E, 260), dtype=np.float32)
        bvaug = np.zeros((1, 260), dtype=np.float32)
        for h in range(4):
            wvaug[:, 65 * h : 65 * h + 64] = wv_s[:, 64 * h : 64 * h + 64]
            bvaug[0, 65 * h : 65 * h + 64] = bv_s[64 * h : 64 * h + 64]
            bvaug[0, 65 * h + 64] = 1.0
        def arr_w(w):
            # [1024, C] -> [128, 8*C]: partition p holds e-chunks j at cols j*C
            C = w.shape[1]
            return np.ascontiguousarray(
                w.reshape(8, 128, C).transpose(1, 0, 2).reshape(128, 8 * C)
            ).astype(bf)

        wo_s = Wo[cols, :]
        cbf_h = np.zeros((128, 516), dtype=btri.dtype)
        cbf_h[:, 0:256] = btri
        cbf_h[0, 256:516] = bvaug.astype(bf)[0]
        cf32_h = np.zeros((128, 4), dtype=np.float32)
        cf32_h[:, 0] = bq[cols][0:128]
        cf32_h[:, 1] = bq[cols][128:256]
        cf32_h[:, 2] = bk[cols][0:128]
        cf32_h[:, 3] = bk[cols][128:256]
        in_maps.append(
            {
                "xt": xtb[b],
                "yt": ytb[b],
                "wq": arr_w(Wq[:, cols]),
                "wk": arr_w(Wk[:, cols]),
                "wvaug": arr_w(wvaug),
                "wo": np.ascontiguousarray(
                    wo_s.reshape(2, 128, E).transpose(1, 0, 2).reshape(128, 2 * E)
                ).astype(bf),
                "cbf": cbf_h,
                "cf32": cf32_h,
            }
        )

    nc = _get_nc()
    res = run_bass_kernel_spmd(
        nc, in_maps, core_ids=list(range(N_CORES)), trace=trace
    )
    LAST_RESULT = res

    full = np.zeros((B, T, E), dtype=np.float32)
    for c in range(N_CORES):
        full[c // 4] += res.results[c]["out"].astype(np.float32)
    full += bo
    return full



# revision 34
# speedup vs baseline: 1.0130x; 1.0130x over previous
"""Distributed Trainium2 Bass kernel for multi-head causal cross-attention.

Reference computation (B=2, T=2048, E=1024, H=16, d=64):
    q = x @ Wq + bq ; k = y @ Wk + bk ; v = y @ Wv + bv      (per-head reshape)
    att = softmax(q k^T / sqrt(d) + causal_mask)
    out = (att v) @ Wo + bo

Sharding over 8 NeuronCores: data-parallel on batch (2 groups of 4 cores),
tensor-parallel on heads (4 heads = 256 channels per core).  Each core
computes a partial output projection; the host sums the 4 partials per batch
(the unshard step for tensor-parallel partial sums) and adds the output bias.
No on-device collectives are needed.

Per-core dataflow (bf16 operands, fp32 PSUM accumulation; measured
rel err ~5.4e-3 vs the fp32 reference):
  - host passes x^T / y^T (bf16) pre-blocked by 512-col t-blocks so every
    input DMA reads 8KB/partition contiguous runs; no on-chip transposes
  - Q^T,K^T = W^T x^T (W stationary), evicted bf16 with fused bias add
  - V in an augmented layout [tk, 4*65]: per head 64 value columns plus a
    ones column, so the PV matmul (M=65) also emits the softmax denominator
    as PSUM row 64
  - scores computed transposed (S^T: tk on partitions, tq free) into a
    2-bank PSUM tile holding both heads of a pair (the two 64-row score
    matmuls run CONCURRENTLY via PE row-group tiling); causal blocks
    skipped; one exp (scale=1/8 fused, no max-subtraction) covers both
    heads via a segmented AP; diagonal 128-blocks masked with one
    segmented 0/1 triangular multiply on the DVE
  - normalization: fast approximate reciprocal of the sums row + gpsimd
    partition-broadcast + fused multiply while evicting A^T
  - one global chunk stream over (J, pair, tk-chunk) with a one-chunk
    score LOOKAHEAD: chunk c+1's score matmuls are queued before exp(c) so
    the scalar engine (the pacer, ~93us of exp) never waits; deferred
    projection work (K/Q/V for J+1, outproj) fills the tensor engine
    between chunks, rebalanced toward the late exp-paced streams
  - PSUM: 2 x 2-bank score slots + 1-bank deferred-work slot x2 + 2
    PV-accumulator banks = 8

Hardware notes baked in (learned from profiling):
  - bf16 moving operands stream 1 col/cycle (2.0-2.4 GHz, power-state
    dependent); all matmul operands bf16, fp32 PSUM accumulation
  - ACT: ~(N+352)/1.2 ns per activation; the one-time exp table load
    (~2.7us) is pulled off the critical path by an early dummy exp
  - HWDGE DMA trigger instructions BLOCK the issuing engine for the whole
    transfer: scalar queue only carries pre-exp loads; sync carries the
    rest; SWDGE (~38 GB/s) only wo
  - gpsimd cannot read PSUM; reciprocal_approx_fast needs an SBUF source
  - HAM clock gate: dense MM stream from first MM on keeps the PE at
    2.4 GHz; any >=3.4us tensor idle window halves the clock
"""

import sys

if "/opt/trn_rl_repo" not in sys.path:
    sys.path.insert(0, "/opt/trn_rl_repo")

import numpy as np
import ml_dtypes

import concourse.bacc as bacc
import concourse.mybir as mybir
import concourse.tile as tile
from concourse.bass_utils import run_bass_kernel_spmd

BF16 = mybir.dt.bfloat16
F32 = mybir.dt.float32
AF = mybir.ActivationFunctionType

B, T, E, H = 2, 2048, 1024, 16
D = E // H                  # 64 head dim
N_CORES = 8
CPC = E // 4                # 256 channels per core (4 heads)
NEG = -1.0e10

_CACHE = {}
LAST_RESULT = None


def _build():
    nc = bacc.Bacc("TRN2", target_bir_lowering=False, debug=False, num_devices=N_CORES)

    # xt/yt are host-pre-blocked: row (J*128 + p), col (j*512 + f) holds
    # x^T[j*128 + p, 512*J + f] -- each 512-col t-block is a contiguous
    # 8 KB/partition DRAM read (1 KB runs measured only ~125 GB/s)
    xt = nc.dram_tensor("xt", [512, 4096], BF16, kind="ExternalInput").ap()
    yt = nc.dram_tensor("yt", [512, 4096], BF16, kind="ExternalInput").ap()
    wq = nc.dram_tensor("wq", [128, 8 * CPC], BF16, kind="ExternalInput").ap()
    wk = nc.dram_tensor("wk", [128, 8 * CPC], BF16, kind="ExternalInput").ap()
    wvaug = nc.dram_tensor("wvaug", [128, 8 * 260], BF16, kind="ExternalInput").ap()
    wo = nc.dram_tensor("wo", [128, 2 * E], BF16, kind="ExternalInput").ap()
    # packed consts: cbf = [btri2 (256) | row0: bvaug (260)], cf32 = [bq0,
    # bq1, bk0, bk1] columns -- one DMA trigger each (HWDGE triggers cost
    # ~700ns of engine time apiece)
    cbf = nc.dram_tensor("cbf", [128, 516], BF16, kind="ExternalInput").ap()
    cf32 = nc.dram_tensor("cf32", [128, 4], F32, kind="ExternalInput").ap()
    out = nc.dram_tensor("out", [T, E], BF16, kind="ExternalOutput").ap()

    with tile.TileContext(nc) as tc:
        with (
            nc.allow_low_precision(reason="f32r intermediates; verified <2e-2 end-to-end"),
            tc.tile_pool(name="big", bufs=1) as big,
            tc.tile_pool(name="pt", bufs=8) as ptp,
            tc.tile_pool(name="small", bufs=3) as sm,
            tc.tile_pool(name="zout", bufs=4) as zp,
        ):
            # ---- inputs stream t-block-first on the two HWDGE queues
            # (sync: yT + wv; scalar: wk/wq + xT + wo) so the first K matmul
            # can start as soon as wk + yT block 0 land (~14us); weights ride
            # HWDGE too -- SWDGE measured only ~38 GB/s.  SWDGE keeps just
            # the tiny consts. ----
            wk_b = big.tile([128, 8 * CPC], BF16, tag="wk_b", name="wk_b")
            wv_b = big.tile([128, 8 * 260], BF16, tag="wv_b", name="wv_b")
            wq_b = big.tile([128, 8 * CPC], BF16, tag="wq_b", name="wq_b")
            wo_b = big.tile([128, 2 * E], BF16, tag="wo_b", name="wo_b")
            yTb = big.tile([128, 8 * T], BF16, tag="yTb", name="yTb")
            xTb = big.tile([128, 8 * T], BF16, tag="xTb", name="xTb")

            def in_blk(dst, src, J):
                return (
                    dst[:].rearrange("p (j f) -> p j f", j=8)[
                        :, :, 512 * J : 512 * J + 512
                    ],
                    src[128 * J : 128 * J + 128, :].rearrange(
                        "p (j f) -> p j f", j=8
                    ),
                )

            # HWDGE DMA triggers BLOCK the issuing engine for the whole
            # transfer (measured), so the scalar engine only carries loads
            # that finish before the first exp (~20us); everything later
            # rides sync (idle) or SWDGE (wo: slow queue but time-rich).
            # Tiny consts go FIRST on scalar -- the V-bias broadcast gated
            # the whole V pipeline when they sat behind SWDGE.
            btri2_t = big.tile([128, 256], BF16, tag="btri2", name="btri2")
            bvaug_t = big.tile([1, 260], BF16, tag="bvaug", name="bvaug")
            cf32_t = big.tile([128, 4], F32, tag="cf32", name="cf32")
            bq_t = [cf32_t[:, p : p + 1] for p in range(2)]
            bk_t = [cf32_t[:, 2 + p : 3 + p] for p in range(2)]
            # scalar queue: wk + wq (+2 tiny consts) only -- everything is
            # done by ~17us, before the first exp.  sync: yT0, xT0 (Q before
            # V: the chunk stream starts right after Q p0), then wv and the
            # remaining blocks.  SWDGE: btri2 (needed ~21us) + wo (~95us).
            xb0 = in_blk(xTb, xt, 0)
            yb0 = in_blk(yTb, yt, 0)
            nc.scalar.dma_start(wk_b[:, 0:1024], wk[:, 0:1024])
            nc.sync.dma_start(yb0[0][:, 0:4], yb0[1][:, 0:4])
            nc.scalar.dma_start(bvaug_t[:], cbf[0:1, 256:516])
            nc.scalar.dma_start(cf32_t[:], cf32[:, :])
            nc.gpsimd.dma_start(btri2_t[:], cbf[:, 0:256])
            nc.scalar.dma_start(wk_b[:, 1024:2048], wk[:, 1024:2048])
            nc.sync.dma_start(yb0[0][:, 4:8], yb0[1][:, 4:8])
            nc.scalar.dma_start(wq_b[:, 0:1024], wq[:, 0:1024])
            nc.sync.dma_start(xb0[0][:, 0:4], xb0[1][:, 0:4])
            nc.scalar.dma_start(wq_b[:, 1024:2048], wq[:, 1024:2048])
            nc.sync.dma_start(xb0[0][:, 4:8], xb0[1][:, 4:8])
            # dummy activation on a scratch tile: pulls the one-time
            # ACT_TABLE_LOAD (~2.7us) off the first real exp; last thing the
            # scalar engine does before the exp stream
            dum_t = sm.tile([1, 16], F32, tag="dum", name="dum")
            nc.scalar.activation(dum_t[:], dum_t[:], AF.Exp, scale=0.125)

            # PE pre-warm: ~20 dep-free matmuls on a memset scratch tile fill
            # the otherwise-idle DMA window (~3.5-12.8us) with PE activity so
            # the HAM clock gate flips to 2.4 GHz BEFORE the real matmuls
            # start (saves ~4us of half-clock execution)
            dum2 = big.tile([128, 512], BF16, tag="dum2", name="dum2")
            nc.vector.memset(dum2[:], 0.0)
            nc.sync.dma_start(wv_b[:, 0:1040], wvaug[:, 0:1040])
            nc.sync.dma_start(wv_b[:, 1040:2080], wvaug[:, 1040:2080])
            for J in range(1, 4):
                nc.sync.dma_start(*in_blk(yTb, yt, J))
                nc.sync.dma_start(*in_blk(xTb, xt, J))
            nc.gpsimd.dma_start(wo_b[:], wo[:, :])

            wk_t = [wk_b[:, CPC * e : CPC * e + CPC] for e in range(8)]
            wv_t = [wv_b[:, 260 * e : 260 * e + 260] for e in range(8)]
            wq_t = [wq_b[:, CPC * e : CPC * e + CPC] for e in range(8)]
            wo_t = [wo_b[:, E * p : E * p + E] for p in range(2)]
            yT = [yTb[:, T * e : T * e + T] for e in range(8)]
            xT = [xTb[:, T * e : T * e + T] for e in range(8)]

            # V-bias broadcast (consts loaded early on the scalar queue)
            vb_bc = big.tile([128, 260], BF16, tag="vb_bc", name="vb_bc")
            nc.gpsimd.partition_broadcast(vb_bc[:], bvaug_t[0:1, :])

            KT = [big.tile([128, T], BF16, tag=f"KT{p}", name=f"KT{p}") for p in range(2)]
            QT = [big.tile([128, T], BF16, tag=f"QT{p}", name=f"QT{p}") for p in range(2)]
            AT = [big.tile([128, T], BF16, tag=f"AT{p}", name=f"AT{p}") for p in range(2)]
            V = [big.tile([128, 260], BF16, tag=f"V{c}", name=f"V{c}") for c in range(16)]

            # ---- fused pipeline over tq-blocks J ----
            # Per J: attention chunk loop for both pairs, with the non-exp PE
            # work (K^T/Q^T/V production for J+1, output projection for J-1)
            # interleaved between chunks so the scalar engine (exp) never
            # starves.  J=0's own QKV is a prelude; J=3's outproj is a tail.
            with tc.tile_pool(name="psa", bufs=2, space="PSUM") as psa:
                dups = psa.tile([65, 512], F32, tag="a0", bufs=1, name="dups")
                for _ in range(20):
                    nc.tensor.matmul(
                        dups[:],
                        dum2[:, 0:65],
                        dum2[:, 0:512],
                        start=True,
                        stop=True,
                    )

                # deferred work uses its own 1-bank PSUM tag "w0" so it never
                # rotates through the score slots (sharing collapsed the
                # score lookahead and bubbled the exp stream); emit_kq is
                # split into two 4-matmul halves so work quanta stay under
                # ~1us between score/PV emissions
                def emit_kq_half(J, p, which, half, st):
                    w_t, dst, bias, src_t = (
                        (wk_t, KT, bk_t, yT) if which == "k" else (wq_t, QT, bq_t, xT)
                    )
                    if half == 0:
                        st["ps"] = psa.tile(
                            [128, 512], F32, tag="w0", bufs=2, name="qk"
                        )
                    ps = st["ps"]
                    for e in range(4 * half, 4 * half + 4):
                        nc.tensor.matmul(
                            ps[:],
                            w_t[e][:, 128 * p : 128 * p + 128],
                            src_t[e][:, 512 * J : 512 * J + 512],
                            start=(e == 0),
                            stop=(e == 7),
                        )
                    if half == 1:
                        nc.vector.tensor_scalar_add(
                            dst[p][:, 512 * J : 512 * J + 512], ps[:], bias[p][:, 0:1]
                        )

                def emit_kq(J, p, which, tag="w0", bufs=2):
                    w_t, dst, bias, src_t = (
                        (wk_t, KT, bk_t, yT) if which == "k" else (wq_t, QT, bq_t, xT)
                    )
                    ps = psa.tile([128, 512], F32, tag=tag, bufs=bufs, name="qk")
                    for e in range(8):
                        nc.tensor.matmul(
                            ps[:],
                            w_t[e][:, 128 * p : 128 * p + 128],
                            src_t[e][:, 512 * J : 512 * J + 512],
                            start=(e == 0),
                            stop=(e == 7),
                        )
                    nc.vector.tensor_scalar_add(
                        dst[p][:, 512 * J : 512 * J + 512], ps[:], bias[p][:, 0:1]
                    )

                def emit_v(c):
                    psv = psa.tile([128, 260], F32, tag="w0", bufs=2, name="psv")
                    for e in range(8):
                        nc.tensor.matmul(
                            psv[:],
                            yT[e][:, 128 * c : 128 * c + 128],
                            wv_t[e][:],
                            start=(e == 0),
                            stop=(e == 7),
                        )
                    # bias + per-head ones columns fused into the eviction
                    nc.vector.tensor_add(V[c][:], psv[:], vb_bc[:])

                def emit_outproj(t, tail=False):
                    z = zp.tile([128, E], BF16, tag="z", name="z")
                    for eo in range(2):
                        pz = psa.tile([128, 512], F32, tag="w0", bufs=2, name="pz")
                        nc.tensor.matmul(
                            pz[:],
                            AT[0][:, 128 * t : 128 * t + 128],
                            wo_t[0][:, 512 * eo : 512 * eo + 512],
                            start=True,
                            stop=False,
                        )
                        nc.tensor.matmul(
                            pz[:],
                            AT[1][:, 128 * t : 128 * t + 128],
                            wo_t[1][:, 512 * eo : 512 * eo + 512],
                            start=False,
                            stop=True,
                        )
                        # mid-kernel: both halves on DVE (scalar paces the
                        # exp stream); tail: exp is done, so the idle scalar
                        # engine takes one half
                        if eo == 0:
                            nc.vector.tensor_copy(z[:, 0:512], pz[:])
                        elif tail:
                            nc.scalar.copy(z[:, 512:1024], pz[:])
                        else:
                            nc.vector.tensor_copy(z[:, 512:1024], pz[:])
                    # sync HWDGE: idle after the input load, and ~7x faster
                    # than SWDGE (the last block's eviction is the kernel
                    # tail)
                    nc.sync.dma_start(out[128 * t : 128 * t + 128, :], z[:])

                # prelude: QKV for J=0 -- K^T groups borrow the (still free)
                # PV-accumulator banks so more partial contractions can stay
                # open while the input DMAs drain; V (yT-dependent) before Q
                # so the xT block-0 DMA has time to land
                emit_kq(0, 0, "k", tag="a0", bufs=1)
                emit_kq(0, 1, "k", tag="a1", bufs=1)
                stq = {}
                emit_kq_half(0, 0, "q", 0, stq)
                # bridge dummies: the Q second half waits ~2-3us on the
                # xT0/wq tail transfers; keep the PE busy meanwhile so the
                # HAM clock gate does not re-throttle mid-prelude
                dups2 = psa.tile([65, 512], F32, tag="a1", bufs=1, name="dups2")
                for _ in range(10):
                    nc.tensor.matmul(
                        dups2[:],
                        dum2[:, 0:65],
                        dum2[:, 0:512],
                        start=True,
                        stop=True,
                    )
                emit_kq_half(0, 0, "q", 1, stq)

                # ---- global chunk stream with one-chunk score lookahead:
                # chunk c+1's score matmuls are queued on the tensor engine
                # BEFORE exp(c), so the scalar engine (the pacer) never
                # stalls at pair/J boundaries; deferred projection work is
                # queued between them to fill the tensor engine during exp.
                acc = {}

                def emit_scores(J, p, i):
                    r = i - 4 * J
                    full = r < 0
                    lo = 0 if full else 128 * r
                    tqs = slice(512 * J + lo, 512 * J + 512)
                    s0 = psa.tile([128, 1024], F32, tag="s0", bufs=2, name="s0")
                    nc.tensor.matmul(
                        s0[:, lo:512],
                        KT[p][0:64, 128 * i : 128 * i + 128],
                        QT[p][0:64, tqs],
                        start=True,
                        stop=True,
                    )
                    nc.tensor.matmul(
                        s0[:, 512 + lo : 1024],
                        KT[p][64:128, 128 * i : 128 * i + 128],
                        QT[p][64:128, tqs],
                        start=True,
                        stop=True,
                    )
                    return (J, p, i, s0, lo, full)

                def normalize(J, p, o0, o1, c0=0, c1=512, tail=False):
                    # gpsimd cannot read PSUM, so the denominator rows stage
                    # through SBUF; approx recip also needs an SBUF source.
                    # On the very last pair the exp stream is over, so the
                    # idle scalar engine does the staging copies instead of
                    # the (tail-critical) DVE.
                    w = c1 - c0
                    ro0 = sm.tile([1, 512], F32, tag="ro0", name="ro0")
                    ro1 = sm.tile([1, 512], F32, tag="ro1", name="ro1")
                    if tail:
                        nc.scalar.copy(ro0[0:1, 0:w], o0[64:65, c0:c1])
                        nc.scalar.copy(ro1[0:1, 0:w], o1[64:65, c0:c1])
                    else:
                        nc.vector.tensor_copy(ro0[0:1, 0:w], o0[64:65, c0:c1])
                        nc.vector.tensor_copy(ro1[0:1, 0:w], o1[64:65, c0:c1])
                    re0 = sm.tile([1, 512], F32, tag="re0", name="re0")
                    re1 = sm.tile([1, 512], F32, tag="re1", name="re1")
                    nc.vector.reciprocal_approx_fast(re0[0:1, 0:w], ro0[0:1, 0:w])
                    nc.vector.reciprocal_approx_fast(re1[0:1, 0:w], ro1[0:1, 0:w])
                    bs0 = sm.tile([64, 512], F32, tag="bs0", name="bs0")
                    bs1 = sm.tile([64, 512], F32, tag="bs1", name="bs1")
                    nc.gpsimd.partition_broadcast(bs0[0:64, 0:w], re0[0:1, 0:w])
                    nc.gpsimd.partition_broadcast(bs1[0:64, 0:w], re1[0:1, 0:w])
                    Js = slice(512 * J + c0, 512 * J + c1)
                    nc.vector.tensor_mul(
                        AT[p][0:64, Js], o0[0:64, c0:c1], bs0[0:64, 0:w]
                    )
                    nc.vector.tensor_mul(
                        AT[p][64:128, Js], o1[0:64, c0:c1], bs1[0:64, 0:w]
                    )

                def process(ch):
                    J, p, i, s0, lo, full = ch
                    nchunks = 4 * J + 4
                    pt0 = ptp.tile([128, 1024], BF16, tag="pt0", name="pt0")
                    if full:
                        nc.scalar.activation(pt0[:], s0[:], AF.Exp, scale=0.125)
                    else:
                        s3 = s0[:].rearrange("p (s f) -> p s f", s=2)[:, :, lo:512]
                        p3 = pt0[:].rearrange("p (s f) -> p s f", s=2)[:, :, lo:512]
                        nc.scalar.activation(p3, s3, AF.Exp, scale=0.125)
                        # causal band: zero weights where tk > tq -- both
                        # heads' diagonal blocks in ONE segmented multiply
                        pdg = pt0[:].rearrange("p (s f) -> p s f", s=2)[
                            :, :, lo : lo + 128
                        ]
                        nc.vector.tensor_mul(
                            pdg,
                            pdg,
                            btri2_t[:].rearrange("p (s f) -> p s f", s=2),
                        )
                    if i == 0:
                        acc[p] = (
                            psa.tile([65, 512], F32, tag="a0", bufs=1, name="a0"),
                            psa.tile([65, 512], F32, tag="a1", bufs=1, name="a1"),
                        )
                    o0, o1 = acc[p]
                    h0 = 65 * (2 * p)
                    h1 = 65 * (2 * p + 1)
                    nc.tensor.matmul(
                        o0[0:65, lo:512],
                        V[i][:, h0 : h0 + 65],
                        pt0[:, lo:512],
                        start=(i == 0),
                        stop=(i == nchunks - 1),
                    )
                    nc.tensor.matmul(
                        o1[0:65, lo:512],
                        V[i][:, h1 : h1 + 65],
                        pt0[:, 512 + lo : 1024],
                        start=(i == 0),
                        stop=(i == nchunks - 1),
                    )
                    if i == nchunks - 1:
                        if J == 3 and p == 1:
                            # tail: normalize in column halves so the final
                            # outproj blocks overlap the second half
                            normalize(J, p, o0, o1, 0, 256, tail=True)
                            # 10/11 have no new deps: their matmuls fill the
                            # tensor engine during the normalize chain
                            emit_outproj(10, tail=True)
                            emit_outproj(11, tail=True)
                            emit_outproj(12, tail=True)
                            emit_outproj(13, tail=True)
                            normalize(J, p, o0, o1, 256, 512, tail=True)
                            emit_outproj(14, tail=True)
                            emit_outproj(15, tail=True)
                        else:
                            normalize(J, p, o0, o1)

                pend = None
                for J in range(4):
                    # deferred work: QKV for J+1 when the data allows it;
                    # outproj rebalanced toward the exp-paced late streams
                    # (J=2: blocks 0-3, J=3: blocks 4-11) where the tensor
                    # engine otherwise starves
                    work = []
                    # J=0: its own V blocks and Q p1 lead the work list (PV
                    # and the pair-1 scores trail them by design)
                    if J == 0:
                        for c in range(0, 4):
                            work.append(lambda c=c: emit_v(c))
                        st0 = {}
                        for h in range(2):
                            work.append(
                                lambda h=h, st=st0: emit_kq_half(0, 1, "q", h, st)
                            )
                    # J=2 first produces its own diag-block V (consumed from
                    # chunk 8 onward; items 0-3 complete by ~chunk 5)
                    if J == 2:
                        for c in range(8, 12):
                            work.append(lambda c=c: emit_v(c))
                    if J < 3:
                        for p in range(2):
                            for w in ("k", "q"):
                                st = {}
                                for h in range(2):
                                    work.append(
                                        lambda p=p, J=J, w=w, h=h, st=st:
                                        emit_kq_half(J + 1, p, w, h, st)
                                    )
                    # v(4-7) stay in J0 (needed early in J1); v(8-11) moved
                    # into J2 above; v(12-15) stay in J2
                    if J == 0:
                        for c in range(4, 8):
                            work.append(lambda c=c: emit_v(c))
                    # v(12-15) are J3's diag blocks, consumed from chunk 12
                    # onward -- produced as J3's first work items (done by
                    # ~chunk 6); all movable outproj blocks also go to J3
                    if J == 3:
                        for c in range(12, 16):
                            work.append(lambda c=c: emit_v(c))
                        for t in range(0, 10):
                            work.append(lambda t=t: emit_outproj(t))

                    nchunks = 4 * J + 4
                    nw = len(work)
                    wi = 0
                    for p in range(2):
                        for i in range(nchunks):
                            ch = emit_scores(J, p, i)
                            # fill the tensor queue before the (dependent)
                            # PV of the previous chunk blocks it
                            hi_w = nw * (p * nchunks + i + 1) // (2 * nchunks)
                            while wi < hi_w:
                                work[wi]()
                                wi += 1
                            if pend is not None:
                                process(pend)
                            pend = ch
                process(pend)

    nc.compile()
    return nc


def _get_nc():
    if "nc" not in _CACHE:
        _CACHE["nc"] = _build()
    return _CACHE["nc"]


def _consts():
    if "consts" not in _CACHE:
        bf = ml_dtypes.bfloat16
        btri1 = (
            np.arange(128)[None, :] >= np.arange(128)[:, None]
        ).astype(np.float32).astype(bf)
        btri = np.ascontiguousarray(np.concatenate([btri1, btri1], axis=1))
        _CACHE["consts"] = (btri,)
    return _CACHE["consts"]


def kernel(
    x, y, mask, Wq, bq, Wk, bk, Wv, bv, Wo, bo, num_heads, trace=False
):
    global LAST_RESULT
    assert int(num_heads) == H
    x = np.asarray(x, dtype=np.float32)
    y = np.asarray(y, dtype=np.float32)
    Wq = np.asarray(Wq, dtype=np.float32)
    Wk = np.asarray(Wk, dtype=np.float32)
    Wv = np.asarray(Wv, dtype=np.float32)
    Wo = np.asarray(Wo, dtype=np.float32)
    bq = np.asarray(bq, dtype=np.float32)
    bk = np.asarray(bk, dtype=np.float32)
    bv = np.asarray(bv, dtype=np.float32)
    bo = np.asarray(bo, dtype=np.float32)

    bf = ml_dtypes.bfloat16
    (btri,) = _consts()

    def blk(a):
        # [T, E] -> transposed+t-blocked [4*128, 8*512]: row (J*128+p),
        # col (j*512+f) = a.T[j*128+p, 512*J+f]
        at = a.T.reshape(8, 128, 4, 512)
        return np.ascontiguousarray(
            at.transpose(2, 1, 0, 3).reshape(512, 4096)
        ).astype(bf)

    xtb = [blk(x[b]) for b in range(B)]
    ytb = [blk(y[b]) for b in range(B)]

    in_maps = []
    for c in range(N_CORES):
        b = c // 4
        g = c % 4
        cols = slice(CPC * g, CPC * g + CPC)
        wv_s = Wv[:, cols]
        bv_s = bv[cols]
        wvaug = np.zeros((E, 260), dtype=np.float32)
        bvaug = np.zeros((1, 260), dtype=np.float32)
        for h in range(4):
            wvaug[:, 65 * h : 65 * h + 64] = wv_s[:, 64 * h : 64 * h + 64]
            bvaug[0, 65 * h : 65 * h + 64] = bv_s[64 * h : 64 * h + 64]
            bvaug[0, 65 * h + 64] = 1.0
        def arr_w(w):
            # [1024, C] -> [128, 8*C]: partition p holds e-chunks j at cols j*C
            C = w.shape[1]
            return np.ascontiguousarray(
                w.reshape(8, 128, C).transpose(1, 0, 2).reshape(128, 8 * C)
            ).astype(bf)

        wo_s = Wo[cols, :]
        cbf_h = np.zeros((128, 516), dtype=btri.dtype)
        cbf_h[:, 0:256] = btri
        cbf_h[0, 256:516] = bvaug.astype(bf)[0]
        cf32_h = np.zeros((128, 4), dtype=np.float32)
        cf32_h[:, 0] = bq[cols][0:128]
        cf32_h[:, 1] = bq[cols][128:256]
        cf32_h[:, 2] = bk[cols][0:128]
        cf32_h[:, 3] = bk[cols][128:256]
        in_maps.append(
            {
                "xt": xtb[b],
                "yt": ytb[b],
                "wq": arr_w(Wq[:, cols]),
                "wk": arr_w(Wk[:, cols]),
                "wvaug": arr_w(wvaug),
                "wo": np.ascontiguousarray(
                    wo_s.reshape(2, 128, E).transpose(1, 0, 2).reshape(128, 2 * E)
                ).astype(bf),
                "cbf": cbf_h,
                "cf32": cf32_h,
            }
        )

    nc = _get_nc()
    res = run_bass_kernel_spmd(
        nc, in_maps, core_ids=list(range(N_CORES)), trace=trace
    )
    LAST_RESULT = res

    full = np.zeros((B, T, E), dtype=np.float32)
    for c in range(N_CORES):
        full[c // 4] += res.results[c]["out"].astype(np.float32)
    full += bo
    return full

